# revision 1
# baseline (speedup 1.0000x reference)
"""GCLConv (GNN message passing) Trainium2 kernel — 8-core edge-parallel.

Strategy:
  - Host: sort edges by destination (row); shard by destination node range
    across 8 cores (6272 nodes/core) => no cross-core reduction needed.
  - Device per core: transpose-mode bf16 dma_gather of h[row]/h[col]
    (feature-major [D, e] tiles, zero input transposes), edge MLP as PE
    matmuls with f32 PSUM accumulation, segment-sum via S-matrix matmul
    accumulated in PSUM per 128-node window, then the node MLP + residual.
  - sigmoid(x) = 0.5*tanh(x/2)+0.5 so Silu/Tanh/Copy share one ACT table set.
  - int16 gather indices: col table split in two halves (<32768 rows each);
    edges grouped by (col-half, window) with per-group padding made uniform
    across cores so one SPMD program serves all 8 cores.
"""
import sys

sys.path.insert(0, "/opt/trn_rl_repo")

import numpy as np
import ml_dtypes

import concourse.bass as bass
import concourse.bacc as bacc
import concourse.mybir as mybir
import concourse.tile as tile
from concourse import bass_utils

BF16 = ml_dtypes.bfloat16

N = 50000
E = 800000
D = 128
H = 128
P = 128
NCORES = 8
WIN = 128                  # nodes per aggregation window
NW = 49                    # windows per core
SHARD = WIN * NW           # 6272 nodes per core
NPAD = SHARD * NCORES      # 50176
COL_SPLIT = 25088          # col gather table split (both halves < 32768)
COL_HI = NPAD - COL_SPLIT  # 25088
GB = 32                    # tiles per gather batch (4096 indices)
NORM = 100.0

FP32 = mybir.dt.float32
BF = mybir.dt.bfloat16
I16 = mybir.dt.int16


def _idx_layout(idx_flat: np.ndarray) -> np.ndarray:
    """Pack int16 indices into the SWDGE layout [128, n/16]:
    index i -> partition i%16, col i//16, replicated across 8 groups."""
    n = idx_flat.shape[0]
    assert n % 16 == 0
    arr = idx_flat.reshape(n // 16, 16).T.astype(np.int16)  # [16, n/16]
    return np.tile(arr, (8, 1))                             # [128, n/16]


def _preprocess(h: np.ndarray, edge_index: np.ndarray):
    """Build per-core edge tiles + metadata. Returns host data dict."""
    row = np.asarray(edge_index[0], dtype=np.int64)
    col = np.asarray(edge_index[1], dtype=np.int64)

    core_of = row // SHARD
    half_of = (col >= COL_SPLIT).astype(np.int64)

    # group counts per (core, half, window)
    win_of = (row % SHARD) // WIN
    counts = np.zeros((NCORES, 2, NW), dtype=np.int64)
    np.add.at(counts, (core_of, half_of, win_of), 1)
    tiles_per_group = np.maximum(1, -(-counts // P))        # ceil, min 1
    T_hw = tiles_per_group.max(axis=0)                      # [2, NW] uniform
    NT = int(T_hw.sum())

    # per-core edge ordering: sort by (half, window)
    col_idx = np.empty((NCORES, 128, NT * 8), dtype=np.int16)
    row_idx = np.empty((NCORES, 128, NT * 8), dtype=np.int16)
    rel_row = np.empty((NCORES, 128, NT), dtype=BF16)
    for k in range(NCORES):
        m = core_of == k
        rk, ck, hk, wk = row[m] - k * SHARD, col[m], half_of[m], win_of[m]
        order = np.lexsort((wk, hk))
        rk, ck, hk, wk = rk[order], ck[order], hk[order], wk[order]
        # group boundaries
        cnt = np.zeros((2, NW), dtype=np.int64)
        np.add.at(cnt, (hk, wk), 1)
        rows_l, cols_l, rel_l = [], [], []
        pos = 0
        for hf in range(2):
            for w in range(NW):
                c = int(cnt[hf, w])
                npad_e = int(T_hw[hf, w]) * P - c
                r_g = rk[pos:pos + c]
                c_g = ck[pos:pos + c] - hf * COL_SPLIT
                rel_g = (r_g % WIN).astype(np.float32)
                pos += c
                if npad_e:
                    r_g = np.concatenate([r_g, np.zeros(npad_e, np.int64)])
                    c_g = np.concatenate([c_g, np.zeros(npad_e, np.int64)])
                    rel_g = np.concatenate(
                        [rel_g, np.full(npad_e, 255.0, np.float32)])
                rows_l.append(r_g)
                cols_l.append(c_g)
                rel_l.append(rel_g)
        r_all = np.concatenate(rows_l)
        c_all = np.concatenate(cols_l)
        rel_all = np.concatenate(rel_l)
        assert r_all.shape[0] == NT * P
        col_idx[k] = _idx_layout(c_all.astype(np.int16))
        row_idx[k] = _idx_layout(r_all.astype(np.int16))
        rel_row[k] = rel_all.reshape(NT, P).T.astype(BF16)

    # gather tables
    h_pad = np.zeros((NPAD, D), dtype=np.float32)
    h_pad[:N] = h
    h_bf = h_pad.astype(BF16)
    hA = np.ascontiguousarray(h_bf[:COL_SPLIT])
    hB = np.ascontiguousarray(h_bf[COL_SPLIT:])
    hrow = h_bf.reshape(NCORES, SHARD, D)                   # per-core shard

    # node-phase buffers per core
    hsh = h_pad.reshape(NCORES, NW, WIN, D)
    h_own = np.ascontiguousarray(
        hsh.transpose(0, 2, 1, 3).reshape(NCORES, WIN, NW * D))  # [128, w*128+d]
    hT = np.ascontiguousarray(
        hsh.transpose(0, 3, 1, 2).reshape(NCORES, D, NW * WIN)).astype(BF16)

    return dict(NT=NT, T_hw=T_hw, col_idx=col_idx, row_idx=row_idx,
                rel_row=rel_row, hA=hA, hB=hB, hrow=hrow,
                h_own=h_own.astype(np.float32), hT=hT)


def _build(nc: bass.Bass, NT: int, T_hw: np.ndarray,
           act_silu, act_tanh):
    """Emit the SPMD program. T_hw: [2, NW] tiles per (half, window)."""
    dt = nc.dram_tensor
    hA_t = dt("hA", [COL_SPLIT, D], BF, kind="ExternalInput")
    hB_t = dt("hB", [COL_HI, D], BF, kind="ExternalInput")
    hrow_t = dt("hrow", [SHARD, D], BF, kind="ExternalInput")
    cidx_t = dt("col_idx", [128, NT * 8], I16, kind="ExternalInput")
    ridx_t = dt("row_idx", [128, NT * 8], I16, kind="ExternalInput")
    rel_t = dt("rel_row", [128, NT], BF, kind="ExternalInput")
    hown_t = dt("h_own", [WIN, NW * D], FP32, kind="ExternalInput")
    hT_t = dt("hT", [D, NW * WIN], BF, kind="ExternalInput")
    # weights / consts (replicated)
    eW1t_t = dt("eW1top", [D, H], BF, kind="ExternalInput")
    eW1b_t = dt("eW1bot", [D, H], BF, kind="ExternalInput")
    eW2_t = dt("eW2", [H, H], BF, kind="ExternalInput")
    combo_t = dt("combo", [H, H + 1], BF, kind="ExternalInput")  # [I | aW]
    nW1t_t = dt("nW1top", [D, H], BF, kind="ExternalInput")
    nW1b_t = dt("nW1bot", [H, H], BF, kind="ExternalInput")      # / NORM
    nW2_t = dt("nW2", [H, D], BF, kind="ExternalInput")
    ones_t = dt("ones_row", [1, WIN], BF, kind="ExternalInput")
    nb2_t = dt("nb2_row", [1, D], BF, kind="ExternalInput")
    eb1_t = dt("eb1", [H, 1], FP32, kind="ExternalInput")
    eb2_t = dt("eb2", [H, 1], FP32, kind="ExternalInput")
    nb1_t = dt("nb1", [H, 1], FP32, kind="ExternalInput")
    jconst_t = dt("jconst", [P, WIN], BF, kind="ExternalInput")
    ident_t = dt("ident", [P, P], FP32, kind="ExternalInput")
    ab_t = dt("ab_c", [P, 1], FP32, kind="ExternalInput")  # 0.5*ab per-partition

    out_t = dt("out", [SHARD, D], FP32, kind="ExternalOutput")

    with tile.TileContext(nc) as tc:
        with (
            tc.tile_pool(name="const", bufs=1) as cp,
            tc.tile_pool(name="gather", bufs=3) as gp,
            tc.tile_pool(name="work", bufs=3) as wp,
            tc.tile_pool(name="agg", bufs=1) as ap_,
            tc.tile_pool(name="ps", bufs=2, space="PSUM") as ps,
            tc.tile_pool(name="psagg", bufs=2, space="PSUM") as psg,
        ):
            # --- resident uploads ---
            def up(shape, dtype, src, tag):
                t = cp.tile(shape, dtype, tag=tag)
                nc.sync.dma_start(out=t[:], in_=src[:])
                return t

            cidx = up([128, NT * 8], I16, cidx_t, "cidx")
            ridx = up([128, NT * 8], I16, ridx_t, "ridx")
            rel = up([128, NT], BF, rel_t, "rel")
            h_own = up([WIN, NW * D], FP32, hown_t, "hown")
            hT = up([D, NW * WIN], BF, hT_t, "hT")
            eW1t = up([D, H], BF, eW1t_t, "eW1t")
            eW1b = up([D, H], BF, eW1b_t, "eW1b")
            eW2 = up([H, H], BF, eW2_t, "eW2")
            combo = up([H, H + 1], BF, combo_t, "combo")
            nW1t = up([D, H], BF, nW1t_t, "nW1t")
            nW1b = up([H, H], BF, nW1b_t, "nW1b")
            nW2 = up([H, D], BF, nW2_t, "nW2")
            ones_r = up([1, WIN], BF, ones_t, "ones")
            nb2_r = up([1, D], BF, nb2_t, "nb2")
            eb1 = up([H, 1], FP32, eb1_t, "eb1")
            eb2 = up([H, 1], FP32, eb2_t, "eb2")
            nb1 = up([H, 1], FP32, nb1_t, "nb1")
            jconst = up([P, WIN], BF, jconst_t, "jconst")
            ident = up([P, P], FP32, ident_t, "ident")
            ab_c = up([P, 1], FP32, ab_t, "ab")

            agg = ap_.tile([WIN, NW * H], FP32)  # node-major agg per window

            # --- tile schedule: (half, window) groups; batched gathers ---
            sched = []  # (tile_idx, half, window, first_in_group, last_in_group)
            t_i = 0
            for hf in range(2):
                for w in range(NW):
                    n_t = int(T_hw[hf, w])
                    for i in range(n_t):
                        sched.append((t_i, hf, w, i == 0, i == n_t - 1))
                        t_i += 1
            assert t_i == NT
            TA = int(T_hw[0].sum())  # tiles in half 0

            # batches never cross the half boundary
            batches = []
            for lo, hi in ((0, TA), (TA, NT)):
                b = lo
                while b < hi:
                    batches.append((b, min(GB, hi - b)))
                    b += GB

            bt_of_tile = {}
            for bi, (b0, nb) in enumerate(batches):
                for j in range(nb):
                    bt_of_tile[b0 + j] = (bi, j)

            gtiles = {}

            def emit_batch(bi):
                b0, nb = batches[bi]
                half = 0 if b0 < TA else 1
                src = hA_t if half == 0 else hB_t
                ct = gp.tile([128, GB * P], BF, tag="gcol")
                rt = gp.tile([128, GB * P], BF, tag="grow")
                for (buf, src_ap, idx) in ((ct, src, cidx), (rt, hrow_t, ridx)):
                    nc.gpsimd.dma_gather(
                        out_ap=buf[:, :nb * P].rearrange("p (a n) -> p a n", a=1),
                        in_ap=src_ap[:],
                        idxs_ap=idx[:, b0 * 8:(b0 + nb) * 8],
                        num_idxs=nb * P,
                        num_idxs_reg=nb * P,
                        elem_size=D,
                        transpose=True,
                        single_packet=False,
                    )
                gtiles[bi] = (ct, rt)

            emit_batch(0)
            for (t, hf, w, first, last) in sched:
                bi, j = bt_of_tile[t]
                if j == 0 and bi + 1 < len(batches):
                    emit_batch(bi + 1)
                ct, rt = gtiles[bi]
                es = slice(j * P, (j + 1) * P)

                ps1 = ps.tile([H, P], FP32, space="PSUM", tag="ps1")
                nc.tensor.matmul(ps1[:], lhsT=eW1t[:], rhs=rt[:, es],
                                 start=True, stop=False)
                nc.tensor.matmul(ps1[:], lhsT=eW1b[:], rhs=ct[:, es],
                                 start=False, stop=True)
                m1 = wp.tile([H, P], BF, tag="m1")
                nc.scalar.activation(m1[:], ps1[:], act_silu, bias=eb1[:])

                ps2 = ps.tile([H, P], FP32, space="PSUM", tag="ps2")
                nc.tensor.matmul(ps2[:], lhsT=eW2[:], rhs=m1[:],
                                 start=True, stop=True)
                m2 = wp.tile([H, P], BF, tag="m2")
                nc.scalar.activation(m2[:], ps2[:], act_silu, bias=eb2[:])

                # [m2_edge_major | att_pre] = m2T.T @ [I | aW]
                ps3 = ps.tile([P, H + 1], FP32, space="PSUM", tag="ps3")
                nc.tensor.matmul(ps3[:], lhsT=m2[:], rhs=combo[:],
                                 start=True, stop=True)
                att_t = wp.tile([P, 1], FP32, tag="att_t")
                nc.scalar.activation(att_t[:], ps3[:, H:H + 1], act_tanh,
                                     bias=ab_c[:], scale=0.5)
                att = wp.tile([P, 1], FP32, tag="att")
                nc.scalar.activation(att[:], att_t[:],
                                     mybir.ActivationFunctionType.Copy,
                                     bias=0.5, scale=0.5)
                ef = wp.tile([P, H], BF, tag="ef")
                nc.vector.tensor_scalar_mul(ef[:], ps3[:, :H], att[:])

                S = wp.tile([P, WIN], BF, tag="S")
                nc.vector.tensor_tensor(
                    out=S[:], in0=rel[:, t:t + 1].to_broadcast([P, WIN]),
                    in1=jconst[:], op=mybir.AluOpType.is_equal)

                if first:
                    pagg_cur = psg.tile([WIN, H], FP32, space="PSUM", tag="pagg")
                pagg = pagg_cur
                nc.tensor.matmul(pagg[:], lhsT=S[:], rhs=ef[:],
                                 start=first, stop=last)
                if last:
                    wslice = slice(w * H, (w + 1) * H)
                    if hf == 0:
                        nc.vector.tensor_copy(agg[:, wslice], pagg[:])
                    else:
                        nc.vector.tensor_add(
                            out=agg[:, wslice], in0=agg[:, wslice], in1=pagg[:])

            # --- node phase ---
            for w in range(NW):
                wsl = slice(w * H, (w + 1) * H)
                aggT_ps = ps.tile([H, WIN], FP32, space="PSUM", tag="ps1")
                nc.tensor.transpose(aggT_ps[:], in_=agg[:, wsl], identity=ident[:])
                aggT = wp.tile([H, WIN], BF, tag="m1")
                nc.vector.tensor_copy(aggT[:], aggT_ps[:])

                psn1 = ps.tile([H, WIN], FP32, space="PSUM", tag="ps2")
                nc.tensor.matmul(psn1[:], lhsT=nW1t[:], rhs=hT[:, wsl],
                                 start=True, stop=False)
                nc.tensor.matmul(psn1[:], lhsT=nW1b[:], rhs=aggT[:],
                                 start=False, stop=True)
                y1 = wp.tile([H, WIN], BF, tag="m2")
                nc.scalar.activation(y1[:], psn1[:], act_silu, bias=nb1[:])

                psn2 = ps.tile([WIN, D], FP32, space="PSUM", tag="ps3")
                nc.tensor.matmul(psn2[:], lhsT=y1[:], rhs=nW2[:],
                                 start=True, stop=False)
                nc.tensor.matmul(psn2[:], lhsT=ones_r[:], rhs=nb2_r[:],
                                 start=False, stop=True)
                o_sb = wp.tile([WIN, D], FP32, tag="osb")
                nc.vector.tensor_add(out=o_sb[:], in0=psn2[:], in1=h_own[:, wsl])
                nc.sync.dma_start(out=out_t[w * WIN:(w + 1) * WIN, :], in_=o_sb[:])
    return nc


def _make_in_maps(prep, inputs):
    eW1 = np.asarray(inputs["eW1"], np.float32)
    aW = np.asarray(inputs["aW"], np.float32)
    nW1 = np.asarray(inputs["nW1"], np.float32)
    combo = np.concatenate([np.eye(H, dtype=np.float32),
                            aW.reshape(H, 1)], axis=1)
    jconst = np.broadcast_to(np.arange(WIN, dtype=np.float32)[None, :], (P, WIN))
    common = {
        "hA": prep["hA"], "hB": prep["hB"],
        "eW1top": eW1[:D].astype(BF16), "eW1bot": eW1[D:].astype(BF16),
        "eW2": np.asarray(inputs["eW2"], np.float32).astype(BF16),
        "combo": combo.astype(BF16),
        "nW1top": nW1[:D].astype(BF16),
        "nW1bot": (nW1[D:] / NORM).astype(BF16),
        "nW2": np.asarray(inputs["nW2"], np.float32).astype(BF16),
        "ones_row": np.ones((1, WIN), BF16),
        "nb2_row": np.asarray(inputs["nb2"], np.float32).reshape(1, D).astype(BF16),
        "eb1": np.asarray(inputs["eb1"], np.float32).reshape(H, 1),
        "eb2": np.asarray(inputs["eb2"], np.float32).reshape(H, 1),
        "nb1": np.asarray(inputs["nb1"], np.float32).reshape(H, 1),
        "jconst": np.ascontiguousarray(jconst).astype(BF16),
        "ident": np.eye(P, dtype=np.float32),
        # tanh form: sigmoid(x+ab) = 0.5*tanh(0.5x + 0.5ab) + 0.5
        "ab_c": np.full((P, 1), 0.5 * float(np.asarray(inputs["ab"]).ravel()[0]),
                        dtype=np.float32),
    }
    in_maps = []
    for k in range(NCORES):
        m = dict(common)
        m["hrow"] = np.ascontiguousarray(prep["hrow"][k])
        m["col_idx"] = np.ascontiguousarray(prep["col_idx"][k])
        m["row_idx"] = np.ascontiguousarray(prep["row_idx"][k])
        m["rel_row"] = np.ascontiguousarray(prep["rel_row"][k])
        m["h_own"] = np.ascontiguousarray(prep["h_own"][k])
        m["hT"] = np.ascontiguousarray(prep["hT"][k])
        in_maps.append(m)
    return in_maps


_RUN_KW = {}


def kernel(**inputs) -> np.ndarray:
    h = np.asarray(inputs["h"], np.float32)
    prep = _preprocess(h, np.asarray(inputs["edge_index"]))

    nc = bacc.Bacc("TRN2", target_bir_lowering=False, debug=False,
                   num_devices=NCORES)
    _build(nc, prep["NT"], prep["T_hw"],
           act_silu=mybir.ActivationFunctionType.Silu,
           act_tanh=mybir.ActivationFunctionType.Tanh)
    nc.compile()

    in_maps = _make_in_maps(prep, inputs)
    res = bass_utils.run_bass_kernel_spmd(
        nc, in_maps, core_ids=list(range(NCORES)), **_RUN_KW)
    out = np.empty((NPAD, D), dtype=np.float32)
    for k in range(NCORES):
        out[k * SHARD:(k + 1) * SHARD] = np.asarray(res.results[k]["out"])
    kernel._last_results = res
    return out[:N]

